# revision 4
# baseline (speedup 1.0000x reference)
"""Trainium2 Bass kernel for nn_LinearNet (complex double-linear).

Reference math (N = 4096):
    R_r = x @ W_r^T          R_i = x @ W_i^T
    C_r = W_r^T @ R_r - W_i^T @ R_i
    C_i = W_r^T @ R_i + W_i^T @ R_r
    out = concat([C_r, C_i], axis=1)                    # [N, 2N]

Sharding: core c owns output columns S_c = [c*512, (c+1)*512) of both C_r
and C_i.  Pass 1 computes R[:, S_c] = x @ W[S_c, :]^T from host-transposed
operands; pass 2 computes C[:, S_c] = W^T @ R[:, S_c] from natural layouts.
No inter-core communication.  All matmuls are float32r (FP22 multiply,
full PE speed at free-dim 512, FP32 accumulate in PSUM).
"""

import numpy as np

N = 4096
P = 128
NCORES = 8
SH = N // NCORES  # 512 columns per core
KT = N // P  # 32 contraction tiles
MSUP = 256  # pass-1 output-row super tile (2 PSUM pairs live)
NSUP = 256  # pass-2 output-row super tile (2 PSUM pairs live)

_CACHE = {}


def _build():
    import concourse.mybir as mybir
    import concourse.tile as tile
    from concourse import bacc

    f32 = mybir.dt.float32
    f32r = mybir.dt.float32r

    nc = bacc.Bacc()
    xT = nc.declare_dram_parameter("xT", [N, N], f32r, isOutput=False)
    wrT = nc.declare_dram_parameter("wrT", [N, SH], f32r, isOutput=False)
    wiT = nc.declare_dram_parameter("wiT", [N, SH], f32r, isOutput=False)
    wr = nc.declare_dram_parameter("wr", [N, N], f32r, isOutput=False)
    wi = nc.declare_dram_parameter("wi", [N, N], f32r, isOutput=False)
    out_r = nc.declare_dram_parameter("out_r", [N, SH], f32, isOutput=True)
    out_i = nc.declare_dram_parameter("out_i", [N, SH], f32, isOutput=True)

    with tile.TileContext(nc) as tc:
        with tc.tile_pool(name="rdram", bufs=1, space="DRAM") as rdram:
            rr_d = rdram.tile([KT, P, SH], f32r)
            ri_d = rdram.tile([KT, P, SH], f32r)

            # ---------- pass 1: R[:, S_c] = x @ W[S_c, :]^T ----------
            # matmul: psum[m, n] = sum_kk xT[k*128+kk, m] * wT[k*128+kk, n]
            with (
                tc.tile_pool(name="wt", bufs=1) as wt_pool,
                tc.tile_pool(name="xs", bufs=4) as xs_pool,
                tc.tile_pool(name="ev1", bufs=4) as ev1_pool,
                tc.tile_pool(name="ps1", bufs=4, space="PSUM") as ps1,
            ):
                wrT_sb = wt_pool.tile([P, KT * SH], f32r)
                wiT_sb = wt_pool.tile([P, KT * SH], f32r)
                nc.sync.dma_start(
                    wrT_sb[:].rearrange("kk (kt n) -> kk kt n", kt=KT),
                    wrT[:].rearrange("(kt kk) n -> kk kt n", kk=P),
                )
                nc.sync.dma_start(
                    wiT_sb[:].rearrange("kk (kt n) -> kk kt n", kt=KT),
                    wiT[:].rearrange("(kt kk) n -> kk kt n", kk=P),
                )

                msub = MSUP // P  # 2
                for ms in range(N // MSUP):  # 16
                    acc_r = [ps1.tile([P, SH], f32, tag="ps_r", name=f"accr{_s}") for _s in range(msub)]
                    acc_i = [ps1.tile([P, SH], f32, tag="ps_i", name=f"acci{_s}") for _s in range(msub)]
                    for k in range(KT):
                        xc = xs_pool.tile([P, MSUP], f32r, tag="xc")
                        nc.sync.dma_start(
                            xc[:],
                            xT[k * P : (k + 1) * P, ms * MSUP : (ms + 1) * MSUP],
                        )
                        first, last = k == 0, k == KT - 1
                        for s in range(msub):
                            lhs = xc[:, s * P : (s + 1) * P]
                            nc.tensor.matmul(
                                acc_r[s][:],
                                lhs,
                                wrT_sb[:, k * SH : (k + 1) * SH],
                                start=first,
                                stop=last,
                            )
                            nc.tensor.matmul(
                                acc_i[s][:],
                                lhs,
                                wiT_sb[:, k * SH : (k + 1) * SH],
                                start=first,
                                stop=last,
                            )
                    for s in range(msub):
                        mt = ms * msub + s
                        er = ev1_pool.tile([P, SH], f32r, tag="er")
                        ei = ev1_pool.tile([P, SH], f32r, tag="ei")
                        nc.vector.tensor_copy(er[:], acc_r[s][:])
                        nc.vector.tensor_copy(ei[:], acc_i[s][:])
                        nc.sync.dma_start(rr_d[mt], er[:])
                        nc.sync.dma_start(ri_d[mt], ei[:])

            # ---------- pass 2: C[:, S_c] = W^T @ R (complex) ----------
            # psum_cr[a, b] = sum_j wr[j, a]*rr[j, b] + (-wi[j, a])*ri[j, b]
            # psum_ci[a, b] = sum_j wr[j, a]*ri[j, b] + wi[j, a]*rr[j, b]
            with (
                tc.tile_pool(name="rres", bufs=1) as r_pool,
                tc.tile_pool(name="ws", bufs=6) as ws_pool,
                tc.tile_pool(name="ev2", bufs=4) as ev2_pool,
                tc.tile_pool(name="ps2", bufs=4, space="PSUM") as ps2,
            ):
                rr_sb = r_pool.tile([P, KT * SH], f32r)
                ri_sb = r_pool.tile([P, KT * SH], f32r)
                nc.sync.dma_start(
                    rr_sb[:].rearrange("kk (kt n) -> kk kt n", kt=KT),
                    rr_d[:].rearrange("kt kk n -> kk kt n"),
                )
                nc.sync.dma_start(
                    ri_sb[:].rearrange("kk (kt n) -> kk kt n", kt=KT),
                    ri_d[:].rearrange("kt kk n -> kk kt n"),
                )

                asub = NSUP // P  # 2
                for a in range(N // NSUP):  # 16
                    acc_cr = [ps2.tile([P, SH], f32, tag="ps_cr", name=f"acccr{_s}") for _s in range(asub)]
                    acc_ci = [ps2.tile([P, SH], f32, tag="ps_ci", name=f"accci{_s}") for _s in range(asub)]
                    for j in range(KT):
                        wrb = ws_pool.tile([P, NSUP], f32r, tag="wrb")
                        wib = ws_pool.tile([P, NSUP], f32r, tag="wib")
                        wnb = ws_pool.tile([P, NSUP], f32r, tag="wnb")
                        nc.sync.dma_start(
                            wrb[:],
                            wr[j * P : (j + 1) * P, a * NSUP : (a + 1) * NSUP],
                        )
                        nc.sync.dma_start(
                            wib[:],
                            wi[j * P : (j + 1) * P, a * NSUP : (a + 1) * NSUP],
                        )
                        nc.vector.tensor_scalar_mul(wnb[:], wib[:], -1.0)
                        rrj = rr_sb[:, j * SH : (j + 1) * SH]
                        rij = ri_sb[:, j * SH : (j + 1) * SH]
                        first, last = j == 0, j == KT - 1
                        for s in range(asub):
                            sl = slice(s * P, (s + 1) * P)
                            nc.tensor.matmul(
                                acc_cr[s][:], wrb[:, sl], rrj, start=first, stop=False
                            )
                            nc.tensor.matmul(
                                acc_ci[s][:], wrb[:, sl], rij, start=first, stop=False
                            )
                            nc.tensor.matmul(
                                acc_cr[s][:], wnb[:, sl], rij, start=False, stop=last
                            )
                            nc.tensor.matmul(
                                acc_ci[s][:], wib[:, sl], rrj, start=False, stop=last
                            )
                    for s in range(asub):
                        at = a * asub + s
                        ecr = ev2_pool.tile([P, SH], f32, tag="ecr")
                        eci = ev2_pool.tile([P, SH], f32, tag="eci")
                        nc.vector.tensor_copy(ecr[:], acc_cr[s][:])
                        nc.vector.tensor_copy(eci[:], acc_ci[s][:])
                        nc.sync.dma_start(out_r[at * P : (at + 1) * P, :], ecr[:])
                        nc.sync.dma_start(out_i[at * P : (at + 1) * P, :], eci[:])

    nc.finalize()
    return nc


def _get_nc():
    if "nc" not in _CACHE:
        _CACHE["nc"] = _build()
    return _CACHE["nc"]


def kernel(x, W_r, W_i, **run_kwargs):
    from concourse.bass_utils import run_bass_kernel_spmd

    x = np.ascontiguousarray(np.asarray(x, dtype=np.float32))
    W_r = np.ascontiguousarray(np.asarray(W_r, dtype=np.float32))
    W_i = np.ascontiguousarray(np.asarray(W_i, dtype=np.float32))

    nc = _get_nc()
    xT = np.ascontiguousarray(x.T)
    in_maps = []
    for c in range(NCORES):
        sl = slice(c * SH, (c + 1) * SH)
        in_maps.append(
            {
                "xT": xT,
                "wrT": np.ascontiguousarray(W_r[sl].T),
                "wiT": np.ascontiguousarray(W_i[sl].T),
                "wr": W_r,
                "wi": W_i,
            }
        )
    out = run_bass_kernel_spmd(nc, in_maps, list(range(NCORES)), **run_kwargs)
    res = out.results

    full = np.empty((N, 2 * N), dtype=np.float32)
    for c in range(NCORES):
        full[:, c * SH : (c + 1) * SH] = res[c]["out_r"]
        full[:, N + c * SH : N + (c + 1) * SH] = res[c]["out_i"]
    if run_kwargs:
        _CACHE["last_result"] = out
    return full


# revision 5
# speedup vs baseline: 1.1449x; 1.1449x over previous
"""Trainium2 Bass kernel for nn_LinearNet (complex double-linear).

Reference math (N = 4096):
    R_r = x @ W_r^T          R_i = x @ W_i^T
    C_r = W_r^T @ R_r - W_i^T @ R_i
    C_i = W_r^T @ R_i + W_i^T @ R_r
    out = concat([C_r, C_i], axis=1)                    # [N, 2N]

Sharding: core c owns output columns S_c = [c*512, (c+1)*512) of both C_r
and C_i.  Pass 1 computes R[:, S_c] = x @ W[S_c, :]^T from host-transposed
operands; pass 2 computes C[:, S_c] = W^T @ R[:, S_c] from natural layouts.
No inter-core communication.  All matmuls are float32r (FP22 multiply,
full PE speed at free-dim 512, FP32 accumulate in PSUM).
"""

import numpy as np

N = 4096
P = 128
NCORES = 8
SH = N // NCORES  # 512 columns per core
KT = N // P  # 32 contraction tiles
MSUP = 256  # pass-1 output-row super tile (2 PSUM pairs live)
NSUP = 256  # pass-2 output-row super tile (2 PSUM pairs live)

_CACHE = {}


def _build():
    import concourse.mybir as mybir
    import concourse.tile as tile
    from concourse import bacc

    f32 = mybir.dt.float32
    f32r = mybir.dt.float32r

    nc = bacc.Bacc()
    xT = nc.declare_dram_parameter("xT", [N, N], f32r, isOutput=False)
    wrT = nc.declare_dram_parameter("wrT", [N, SH], f32r, isOutput=False)
    wiT = nc.declare_dram_parameter("wiT", [N, SH], f32r, isOutput=False)
    wr = nc.declare_dram_parameter("wr", [N, N], f32r, isOutput=False)
    wi = nc.declare_dram_parameter("wi", [N, N], f32r, isOutput=False)
    out_r = nc.declare_dram_parameter("out_r", [N, SH], f32, isOutput=True)
    out_i = nc.declare_dram_parameter("out_i", [N, SH], f32, isOutput=True)

    with tile.TileContext(nc) as tc:
        with tc.tile_pool(name="rdram", bufs=1, space="DRAM") as rdram:
            rr_d = [
                rdram.tile([P, SH], f32r, name=f"rrd{m}", tag=f"rrd{m}")
                for m in range(KT)
            ]
            ri_d = [
                rdram.tile([P, SH], f32r, name=f"rid{m}", tag=f"rid{m}")
                for m in range(KT)
            ]

            # ---------- pass 1: R[:, S_c] = x @ W[S_c, :]^T ----------
            # matmul: psum[m, n] = sum_kk xT[k*128+kk, m] * wT[k*128+kk, n]
            with (
                tc.tile_pool(name="wt", bufs=1) as wt_pool,
                tc.tile_pool(name="xs", bufs=8) as xs_pool,
                tc.tile_pool(name="ev1", bufs=4) as ev1_pool,
                tc.tile_pool(name="ps1", bufs=4, space="PSUM") as ps1,
            ):
                wrT_sb = wt_pool.tile([P, KT * SH], f32r)
                wiT_sb = wt_pool.tile([P, KT * SH], f32r)
                nc.scalar.dma_start(
                    wrT_sb[:].rearrange("kk (kt n) -> kk kt n", kt=KT),
                    wrT[:].rearrange("(kt kk) n -> kk kt n", kk=P),
                )
                nc.scalar.dma_start(
                    wiT_sb[:].rearrange("kk (kt n) -> kk kt n", kt=KT),
                    wiT[:].rearrange("(kt kk) n -> kk kt n", kk=P),
                )

                msub = MSUP // P  # 2
                for ms in range(N // MSUP):  # 16
                    acc_r = [ps1.tile([P, SH], f32, tag="ps_r", name=f"accr{_s}") for _s in range(msub)]
                    acc_i = [ps1.tile([P, SH], f32, tag="ps_i", name=f"acci{_s}") for _s in range(msub)]
                    for k in range(KT):
                        xc = xs_pool.tile([P, MSUP], f32r, tag="xc")
                        nc.sync.dma_start(
                            xc[:],
                            xT[k * P : (k + 1) * P, ms * MSUP : (ms + 1) * MSUP],
                        )
                        first, last = k == 0, k == KT - 1
                        for s in range(msub):
                            lhs = xc[:, s * P : (s + 1) * P]
                            nc.tensor.matmul(
                                acc_r[s][:],
                                lhs,
                                wrT_sb[:, k * SH : (k + 1) * SH],
                                start=first,
                                stop=last,
                            )
                            nc.tensor.matmul(
                                acc_i[s][:],
                                lhs,
                                wiT_sb[:, k * SH : (k + 1) * SH],
                                start=first,
                                stop=last,
                            )
                    for s in range(msub):
                        mt = ms * msub + s
                        er = ev1_pool.tile([P, SH], f32r, tag="er")
                        ei = ev1_pool.tile([P, SH], f32r, tag="ei")
                        nc.vector.tensor_copy(er[:], acc_r[s][:])
                        nc.vector.tensor_copy(ei[:], acc_i[s][:])
                        nc.gpsimd.dma_start(rr_d[mt][:], er[:])
                        nc.gpsimd.dma_start(ri_d[mt][:], ei[:])

            # ---------- pass 2: C[:, S_c] = W^T @ R (complex) ----------
            # psum_cr[a, b] = sum_j wr[j, a]*rr[j, b] + (-wi[j, a])*ri[j, b]
            # psum_ci[a, b] = sum_j wr[j, a]*ri[j, b] + wi[j, a]*rr[j, b]
            with (
                tc.tile_pool(name="rres", bufs=1) as r_pool,
                tc.tile_pool(name="ws", bufs=8) as ws_pool,
                tc.tile_pool(name="rn", bufs=4) as rn_pool,
                tc.tile_pool(name="ev2", bufs=4) as ev2_pool,
                tc.tile_pool(name="ps2", bufs=4, space="PSUM") as ps2,
            ):
                rr_sb = r_pool.tile([P, KT * SH], f32r)
                ri_sb = r_pool.tile([P, KT * SH], f32r)
                for m in range(KT):
                    nc.scalar.dma_start(
                        rr_sb[:, m * SH : (m + 1) * SH], rr_d[m][:]
                    )
                    nc.scalar.dma_start(
                        ri_sb[:, m * SH : (m + 1) * SH], ri_d[m][:]
                    )

                asub = NSUP // P  # 2
                for a in range(N // NSUP):  # 16
                    acc_cr = [ps2.tile([P, SH], f32, tag="ps_cr", name=f"acccr{_s}") for _s in range(asub)]
                    acc_ci = [ps2.tile([P, SH], f32, tag="ps_ci", name=f"accci{_s}") for _s in range(asub)]
                    for j in range(KT):
                        wrb = ws_pool.tile([P, NSUP], f32r, tag="wrb")
                        wib = ws_pool.tile([P, NSUP], f32r, tag="wib")
                        wnb = ws_pool.tile([P, NSUP], f32r, tag="wnb")
                        nc.sync.dma_start(
                            wrb[:],
                            wr[j * P : (j + 1) * P, a * NSUP : (a + 1) * NSUP],
                        )
                        nc.sync.dma_start(
                            wib[:],
                            wi[j * P : (j + 1) * P, a * NSUP : (a + 1) * NSUP],
                        )
                        nc.vector.tensor_scalar_mul(wnb[:], wib[:], -1.0)
                        rrj = rr_sb[:, j * SH : (j + 1) * SH]
                        rij = ri_sb[:, j * SH : (j + 1) * SH]
                        rnj = rn_pool.tile([P, SH], f32r, tag="rnj", name="rnj")
                        nc.vector.tensor_scalar_mul(rnj[:], rrj, -1.0)
                        first, last = j == 0, j == KT - 1
                        for s in range(asub):
                            sl = slice(s * P, (s + 1) * P)
                            nc.tensor.matmul(
                                acc_cr[s][:], wrb[:, sl], rrj, start=first, stop=False
                            )
                            nc.tensor.matmul(
                                acc_ci[s][:], wrb[:, sl], rij, start=first, stop=False
                            )
                            nc.tensor.matmul(
                                acc_cr[s][:], wnb[:, sl], rij, start=False, stop=last
                            )
                            nc.tensor.matmul(
                                acc_ci[s][:], wnb[:, sl], rnj[:], start=False, stop=last
                            )
                    for s in range(asub):
                        at = a * asub + s
                        ecr = ev2_pool.tile([P, SH], f32, tag="ecr")
                        eci = ev2_pool.tile([P, SH], f32, tag="eci")
                        nc.scalar.copy(ecr[:], acc_cr[s][:])
                        nc.scalar.copy(eci[:], acc_ci[s][:])
                        nc.gpsimd.dma_start(out_r[at * P : (at + 1) * P, :], ecr[:])
                        nc.gpsimd.dma_start(out_i[at * P : (at + 1) * P, :], eci[:])

    nc.finalize()
    return nc


def _get_nc():
    if "nc" not in _CACHE:
        _CACHE["nc"] = _build()
    return _CACHE["nc"]


def kernel(x, W_r, W_i, **run_kwargs):
    from concourse.bass_utils import run_bass_kernel_spmd

    x = np.ascontiguousarray(np.asarray(x, dtype=np.float32))
    W_r = np.ascontiguousarray(np.asarray(W_r, dtype=np.float32))
    W_i = np.ascontiguousarray(np.asarray(W_i, dtype=np.float32))

    nc = _get_nc()
    xT = np.ascontiguousarray(x.T)
    in_maps = []
    for c in range(NCORES):
        sl = slice(c * SH, (c + 1) * SH)
        in_maps.append(
            {
                "xT": xT,
                "wrT": np.ascontiguousarray(W_r[sl].T),
                "wiT": np.ascontiguousarray(W_i[sl].T),
                "wr": W_r,
                "wi": W_i,
            }
        )
    out = run_bass_kernel_spmd(nc, in_maps, list(range(NCORES)), **run_kwargs)
    res = out.results

    full = np.empty((N, 2 * N), dtype=np.float32)
    for c in range(NCORES):
        full[:, c * SH : (c + 1) * SH] = res[c]["out_r"]
        full[:, N + c * SH : N + (c + 1) * SH] = res[c]["out_i"]
    if run_kwargs:
        _CACHE["last_result"] = out
    return full
